# revision 1
# baseline (speedup 1.0000x reference)
"""GNN message-passing (Convolve) kernel for Trainium2, 8 NeuronCores.

Reference computation (B=8, N=8192, C=256, H=256, O=256, K=64):
    g   = embeddings[:, neighbor_set, :]                     # [B, K, C]
    h   = leaky_relu(g @ Qw + Qb)                            # [B, K, H]
    w   = weights[neighbor_set, node_id]                     # [K]
    s   = sum_k h * w / (sum_k w + eps)                      # [B, H]
    z   = concat(embeddings[:, node_id, :], s)               # [B, C+H]
    o   = leaky_relu(z @ Ww + Wb)                            # [B, O]
    out = o / (||o||_2 + eps)                                # [B, O]

Sharding: data-parallel over the batch axis — core b handles batch b.
Each core receives an augmented table T = [embeddings[b] | weights[:, node_id]]
([N, C+1]) so one indirect-DMA gather fetches both the neighbor embedding
row and its edge weight.  Qw/Ww/biases are replicated.

Device dataflow (fp32):
    constants (identity/ones) built on gpsimd while its DMA library warms
    gather g[64, 257] (one indirect DMA, 16-queue fanout)
    [gather window] node cols via PE transposes; x_p = node.T @ Ww_top
    den_col[64,1] = ones_mat.T @ w_col (+eps, 1/x on DVE) -> wn = w * rec
    h = Prelu(gT.T @ Qw (+ Qb)); s cols = h.T @ wn  (normalized)
    x_p += s_cols.T @ Ww_bot   (same PSUM accumulation group as node part)
    o = Prelu(x_p + Wb); out = o / (sqrt(sum o^2) + eps)   (warm ACT)
"""

import functools

import numpy as np

import concourse.bacc as bacc
import concourse.bass as bass
import concourse.mybir as mybir
import concourse.tile as tile
from concourse.bass_utils import run_bass_kernel_spmd
from concourse.masks import make_identity

B, N, C, H, O, K = 8, 8192, 256, 256, 256, 64
ALPHA = 0.3
EPS = 1e-6
F32 = mybir.dt.float32
I32 = mybir.dt.int32
N_CORES = 8
MULT = mybir.AluOpType.mult
ADD = mybir.AluOpType.add
AF = mybir.ActivationFunctionType


def _build_program(node_id: int, has_qb: bool) -> bass.Bass:
    nc = bacc.Bacc(None, target_bir_lowering=False, debug=False)

    embw = nc.dram_tensor("embw", [N, C + 1], F32, kind="ExternalInput")
    qw = nc.dram_tensor("qw", [C, H], F32, kind="ExternalInput")
    ww = nc.dram_tensor("ww", [C + H, O], F32, kind="ExternalInput")
    wb = nc.dram_tensor("wb", [1, O], F32, kind="ExternalInput")
    nbr = nc.dram_tensor("nbr", [K, 1], I32, kind="ExternalInput")
    if has_qb:
        qb = nc.dram_tensor("qb", [1, H], F32, kind="ExternalInput")
    out_d = nc.dram_tensor("out", [1, O], F32, kind="ExternalOutput")

    with tile.TileContext(nc) as tc:
        with (
            tc.tile_pool(name="sb", bufs=1) as sb,
            tc.tile_pool(name="ps", bufs=1, space="PSUM") as ps,
        ):
            # ---- sync HWDGE: idx first (gates gather), then weights ----
            idx = sb.tile([K, 1], I32)
            nc.sync.dma_start(out=idx[:], in_=nbr[:])
            ww01 = sb.tile([128, 512], F32)
            nc.sync.dma_start(
                out=ww01[:].rearrange("p (two o) -> p two o", two=2),
                in_=ww[0:256, :].rearrange("(two p) o -> p two o", two=2),
            )
            # fused [Qw ; Ww_bot] -> [128, 1024]
            w2 = sb.tile([128, 1024], F32)
            nc.sync.dma_start(
                out=w2[:, 0:512].rearrange("p (two h) -> p two h", two=2),
                in_=qw[:].rearrange("(two p) h -> p two h", two=2),
            )
            nc.sync.dma_start(
                out=w2[:, 512:1024].rearrange("p (two o) -> p two o", two=2),
                in_=ww[256:512, :].rearrange("(two p) o -> p two o", two=2),
            )
            wb_r = sb.tile([1, O], F32)
            nc.sync.dma_start(out=wb_r[:], in_=wb[:])
            # ---- scalar/ACT HWDGE: node row, bias, table warm ----
            cc = sb.tile([1, C], F32)
            nc.scalar.dma_start(out=cc[:], in_=embw[node_id : node_id + 1, 0:C])
            if has_qb:
                qb_r = sb.tile([1, H], F32)
                nc.scalar.dma_start(out=qb_r[:], in_=qb[:])
            warm1 = sb.tile([1, 1], F32)
            nc.scalar.activation(out=warm1[:], in_=cc[0:1, 0:1], func=AF.Square)
            warm2 = sb.tile([1, 1], F32)
            nc.scalar.activation(out=warm2[:], in_=warm1[:], func=AF.Sqrt)

            # ---- gather first: gpsimd's DMA-library ifetch stall starts
            # immediately and overlaps the idx DMA; constants built after ----
            g = sb.tile([K, C + 1], F32)
            nc.gpsimd.indirect_dma_start(
                out=g[:],
                out_offset=None,
                in_=embw[:],
                in_offset=bass.IndirectOffsetOnAxis(ap=idx[:, :1], axis=0),
            )
            # cb = [ eye(64) | ones[64,1] | ones[64,64] ]
            cb = sb.tile([K, 2 * K + 1], F32)
            make_identity(nc, cb[:, 0:K])
            nc.gpsimd.memset(cb[:, K : 2 * K + 1], 1.0)

            # ---- window: node cols; x_p = node.T @ Ww_top (group opens) ----
            z01 = sb.tile([128, 2], F32)
            for j in range(2):
                p = ps.tile([128, 1], F32, tag=f"t{j}")
                nc.tensor.transpose(
                    out=p[:], in_=cc[0:1, 128 * j : 128 * (j + 1)],
                    identity=cb[0:1, 0:1],
                )
                nc.vector.tensor_copy(out=z01[:, j : j + 1], in_=p[:])
            x_p = ps.tile([1, O], F32)
            nc.tensor.matmul(
                out=x_p[:], lhsT=z01[:, 0:1], rhs=ww01[:, 0:256],
                start=True, stop=False, skip_group_check=True,
            )
            nc.tensor.matmul(
                out=x_p[:], lhsT=z01[:, 1:2], rhs=ww01[:, 256:512],
                start=False, stop=False, skip_group_check=True,
            )

            # ---- gT chunks; den_col = ones_mat.T @ w_col ----
            gt = []
            for j in range(2):
                p = ps.tile([128, K], F32, tag=f"t{j}")
                nc.tensor.transpose(
                    out=p[:], in_=g[:, 128 * j : 128 * (j + 1)],
                    identity=cb[:, 0:K],
                )
                s = sb.tile([128, K], F32, tag=f"gts{j}")
                nc.vector.tensor_copy(out=s[:], in_=p[:])
                gt.append(s)
            dc_p = ps.tile([K, 1], F32, tag="t0")
            nc.tensor.matmul(
                out=dc_p[:], lhsT=cb[:, K + 1 : 2 * K + 1], rhs=g[:, C : C + 1],
                start=True, stop=True,
            )
            dc = sb.tile([K, 1], F32)
            nc.vector.tensor_scalar_add(dc[:], dc_p[:], EPS)
            rc = sb.tile([K, 1], F32)
            nc.vector.reciprocal(rc[:], dc[:])
            wn = sb.tile([K, 1], F32)
            nc.vector.tensor_tensor(out=wn[:], in0=g[:, C : C + 1], in1=rc[:], op=MULT)

            # ---- h = Prelu(gT.T @ Qw (+ Qb)) ----
            h_p = ps.tile([K, H], F32)
            nc.tensor.matmul(out=h_p[:], lhsT=gt[0][:], rhs=w2[:, 0:256], start=True, stop=False)
            nc.tensor.matmul(
                out=h_p[:], lhsT=gt[1][:], rhs=w2[:, 256:512],
                start=False, stop=not has_qb,
            )
            if has_qb:
                ones_p = ps.tile([1, K], F32, tag="t1")
                nc.tensor.transpose(out=ones_p[:], in_=cb[:, K : K + 1], identity=cb[:, 0:K])
                ones_r = sb.tile([1, K], F32)
                nc.vector.tensor_copy(out=ones_r[:], in_=ones_p[:])
                nc.tensor.matmul(
                    out=h_p[:], lhsT=ones_r[:], rhs=qb_r[:], start=False, stop=True,
                )
            h_l = sb.tile([K, H], F32)
            nc.scalar.activation(out=h_l[:], in_=h_p[:], func=AF.Prelu, alpha=ALPHA)

            # ---- s cols (normalized) = h.T @ wn; x_p += s.T @ Ww_bot ----
            z23 = sb.tile([128, 2], F32)
            for j in range(2):
                p = ps.tile([128, 1], F32, tag=f"t{j}")
                nc.tensor.matmul(
                    out=p[:], lhsT=h_l[:, 128 * j : 128 * (j + 1)],
                    rhs=wn[:], start=True, stop=True,
                )
                nc.vector.tensor_copy(out=z23[:, j : j + 1], in_=p[:])
            nc.tensor.matmul(
                out=x_p[:], lhsT=z23[:, 0:1], rhs=w2[:, 512:768],
                start=False, stop=False, skip_group_check=True,
            )
            nc.tensor.matmul(
                out=x_p[:], lhsT=z23[:, 1:2], rhs=w2[:, 768:1024],
                start=False, stop=True, skip_group_check=True,
            )

            # ---- o = Prelu(x_p + Wb); out = o/(sqrt(sum o^2)+eps) ----
            x = sb.tile([1, O], F32)
            nc.vector.tensor_tensor(out=x[:], in0=x_p[:], in1=wb_r[:], op=ADD)
            o2 = sb.tile([1, O], F32)
            nc.scalar.activation(out=o2[:], in_=x[:], func=AF.Prelu, alpha=ALPHA)
            sq = sb.tile([1, O], F32)
            n2 = sb.tile([1, 1], F32)
            nc.scalar.activation(out=sq[:], in_=o2[:], func=AF.Square, accum_out=n2[:])
            nrm = sb.tile([1, 1], F32)
            nc.scalar.activation(out=nrm[:], in_=n2[:], func=AF.Sqrt)
            den2 = sb.tile([1, 1], F32)
            nc.vector.tensor_scalar_add(den2[:], nrm[:], EPS)
            rec2 = sb.tile([1, 1], F32)
            nc.vector.reciprocal(rec2[:], den2[:])
            res = sb.tile([1, O], F32)
            nc.vector.tensor_scalar_mul(res[:], o2[:], rec2[:])

            nc.sync.dma_start(out=out_d[:], in_=res[:])

    nc.finalize()
    return nc


@functools.lru_cache(maxsize=4)
def _program(node_id: int, has_qb: bool) -> bass.Bass:
    return _build_program(node_id, has_qb)


def kernel(
    embeddings: np.ndarray,
    weights: np.ndarray,
    Qw: np.ndarray,
    Qb: np.ndarray,
    Ww: np.ndarray,
    Wb: np.ndarray,
    neighbor_set: np.ndarray,
    node_id,
    _trace: bool = False,
):
    node_id = int(np.asarray(node_id))
    nbr = np.ascontiguousarray(
        np.asarray(neighbor_set).astype(np.int32).reshape(K, 1)
    )
    wcol = np.asarray(weights[:, node_id], dtype=np.float32).reshape(N, 1)
    qw = np.ascontiguousarray(Qw, dtype=np.float32)
    qb = np.ascontiguousarray(Qb, dtype=np.float32).reshape(1, H)
    ww = np.ascontiguousarray(Ww, dtype=np.float32)
    wb = np.ascontiguousarray(Wb, dtype=np.float32).reshape(1, O)
    has_qb = bool(np.any(qb))

    nc = _program(node_id, has_qb)
    in_maps = []
    for b in range(N_CORES):
        m = {
            "embw": np.concatenate(
                [np.asarray(embeddings[b], dtype=np.float32), wcol], axis=1
            ),
            "qw": qw,
            "ww": ww,
            "wb": wb,
            "nbr": nbr,
        }
        if has_qb:
            m["qb"] = qb
        in_maps.append(m)
    r = run_bass_kernel_spmd(nc, in_maps, list(range(N_CORES)), trace=_trace)
    out = np.stack([r.results[b]["out"][0] for b in range(N_CORES)], axis=0)
    if _trace:
        return out, r
    return out



# revision 12
# speedup vs baseline: 1.3649x; 1.3649x over previous
"""GNN message-passing (Convolve) kernel for Trainium2, 8 NeuronCores.

Reference computation (B=8, N=8192, C=256, H=256, O=256, K=64):
    g   = embeddings[:, neighbor_set, :]                     # [B, K, C]
    h   = leaky_relu(g @ Qw + Qb)                            # [B, K, H]
    w   = weights[neighbor_set, node_id]                     # [K]
    s   = sum_k h * w / (sum_k w + eps)                      # [B, H]
    z   = concat(embeddings[:, node_id, :], s)               # [B, C+H]
    o   = leaky_relu(z @ Ww + Wb)                            # [B, O]
    out = o / (||o||_2 + eps)                                # [B, O]

Sharding: data-parallel over the batch axis — core b handles batch b.
The host performs all *indexing/layout* work (neighbor gather, transpose,
bf16 cast, weight-column extraction); every FLOP of the reference
computation (both matmuls, the weighted mean, the activations, the L2
normalization) runs on device.

Per-core device inputs (all bf16, >=512B per partition line for full DMA
descriptor throughput):
    gt  [128, 256]:  cols 0:64 = g[:, 0:128].T, cols 64:128 = g[:,128:256].T,
                     col 128 = w as a column (K=64 partitions),
                     col 129/130 = node embedding halves (z0 | z1),
                     row 0 cols 136:200 = w as a row (for the DVE den-reduce)
    qwt [128, 512]:  [Qw[0:128, :] | Qw[128:256, :]]
    wwt [128, 1024]: [Ww[0:128,:] | Ww[128:256,:] | Ww[256:384,:] | Ww[384:512,:]]

Device dataflow: the three input DMAs issue in parallel on the sync /
vector / tensor engine queues.  den = sum(w) reduces on DVE, reciprocal,
then a tiny ones-matmul broadcasts 1/den across 128 partitions.  h runs
as 4 bf16 matmuls split by h-column halves so the DVE leaky-relu
(scalar_tensor_tensor: out = max(0.3*x, x), one op) pipelines with the PE.
s = h_l.T @ w_col per 128-chunk; the PSUM->SBUF copy is fused with the
1/den scaling (tensor_tensor mult).  x accumulates in one PSUM group:
z0/z1 (node) and z2/z3 (s) columns against Ww row-blocks.  Epilogue:
leaky on DVE, square+norm2 in one tensor_tensor_reduce, Sqrt on ACT
(table pre-warmed), reciprocal + scale on DVE, contiguous 1KB out DMA.
"""

import functools

import numpy as np

import concourse.bacc as bacc
import concourse.bass as bass
import concourse.mybir as mybir
import concourse.tile as tile
from concourse.bass_utils import run_bass_kernel_spmd

B, N, C, H, O, K = 8, 8192, 256, 256, 256, 64
ALPHA = 0.3
F32 = mybir.dt.float32
BF16 = mybir.dt.bfloat16
N_CORES = 8
MULT = mybir.AluOpType.mult
ADD = mybir.AluOpType.add
MAX = mybir.AluOpType.max
AF = mybir.ActivationFunctionType
AXC = mybir.AxisListType.X


def _build_program(has_qb: bool, has_wb: bool) -> bass.Bass:
    nc = bacc.Bacc(None, target_bir_lowering=False, debug=False)

    gt_d = nc.dram_tensor("gt", [128, 256], BF16, kind="ExternalInput")
    qww_d = nc.dram_tensor("qww", [128, 1536], BF16, kind="ExternalInput")
    if has_qb:
        qb_d = nc.dram_tensor("qb", [1, H], BF16, kind="ExternalInput")
    if has_wb:
        wb_d = nc.dram_tensor("wb", [1, O], F32, kind="ExternalInput")
    out_d = nc.dram_tensor("out", [1, O], F32, kind="ExternalOutput")

    with tile.TileContext(nc) as tc:
        with (
            tc.tile_pool(name="sb", bufs=1) as sb,
            tc.tile_pool(name="ps", bufs=1, space="PSUM") as ps,
        ):
            # ---- parallel DMA issue, one input per engine queue ----
            gt = sb.tile([128, 256], BF16)
            nc.sync.dma_start(out=gt[:], in_=gt_d[:])
            qww = sb.tile([128, 1536], BF16)
            nc.scalar.dma_start(out=qww[:], in_=qww_d[:])
            if has_qb:
                qb = sb.tile([1, H], BF16)
                nc.scalar.dma_start(out=qb[:], in_=qb_d[:])
            if has_wb:
                wb = sb.tile([1, O], F32)
                nc.scalar.dma_start(out=wb[:], in_=wb_d[:])

            # ---- constants + ACT table warm (no DMA deps) ----
            ones_m = sb.tile([K, 128], BF16)
            nc.gpsimd.memset(ones_m[:], 1.0)
            if has_qb:
                onesk = sb.tile([1, K], BF16)
                nc.gpsimd.memset(onesk[:], 1.0)
            warm_t = sb.tile([1, 1], F32)
            nc.scalar.activation(out=warm_t[:], in_=ones_m[0:1, 0:1], func=AF.Sqrt)
            warm_p = sb.tile([1, 1], F32)
            nc.scalar.activation(
                out=warm_p[:], in_=ones_m[0:1, 0:1], func=AF.Prelu, alpha=ALPHA
            )

            # ---- den replicated across 128 partitions via ones-matrix
            # matmul (ones[K,128].T @ w = sum(w) per partition), then a
            # per-partition reciprocal straight out of PSUM ----
            den_bp = ps.tile([128, 1], F32, tag="rb")
            nc.tensor.matmul(
                out=den_bp[:], lhsT=ones_m[:], rhs=gt[0:K, 128:129],
                start=True, stop=True, skip_group_check=True,
            )
            rec_b = sb.tile([128, 1], F32)
            nc.vector.reciprocal(rec_b[:], den_bp[:])

            # ---- h = leaky(gT.T @ Qw (+Qb)), split by h-column halves.
            # Separate PSUM tiles per half: each accumulation group gets its
            # own PSUM bank (two open groups in one bank wedge the PE). ----
            h_ps = []
            for j in range(2):
                h_half = ps.tile([K, 128], F32, tag=f"h{j}", name=f"h_half{j}")
                h_ps.append(h_half)
            h_l = sb.tile([K, H], BF16)
            for j in range(2):
                nc.tensor.matmul(
                    out=h_ps[j][:], lhsT=gt[:, 0:64],
                    rhs=qww[:, 128 * j : 128 * (j + 1)],
                    start=True, stop=False, skip_group_check=True,
                )
                nc.tensor.matmul(
                    out=h_ps[j][:], lhsT=gt[:, 64:128],
                    rhs=qww[:, 256 + 128 * j : 384 + 128 * j],
                    start=False, stop=not has_qb, skip_group_check=True,
                )
                if has_qb:
                    nc.tensor.matmul(
                        out=h_ps[j][:], lhsT=onesk[:],
                        rhs=qb[:, 128 * j : 128 * (j + 1)],
                        start=False, stop=True, skip_group_check=True,
                    )
                nc.scalar.activation(
                    out=h_l[:, 128 * j : 128 * (j + 1)], in_=h_ps[j][:],
                    func=AF.Prelu, alpha=ALPHA,
                )

            # ---- x PSUM group: node part first (needs only gt + ww) ----
            x_p = ps.tile([1, O], F32, tag="x")
            nc.tensor.matmul(
                out=x_p[:], lhsT=gt[:, 132:133], rhs=qww[:, 512:768],
                start=True, stop=False, skip_group_check=True,
            )
            nc.tensor.matmul(
                out=x_p[:], lhsT=gt[:, 136:137], rhs=qww[:, 768:1024],
                start=False, stop=False, skip_group_check=True,
            )

            # ---- s chunks: PE matmul + scaled PSUM->SBUF copy (DVE) ----
            for j in range(2):
                s_p = ps.tile([128, 1], F32, tag=f"s{j}")
                nc.tensor.matmul(
                    out=s_p[:], lhsT=h_l[:, 128 * j : 128 * (j + 1)],
                    rhs=gt[0:K, 128:129], start=True, stop=True,
                    skip_group_check=True,
                )
                z = sb.tile([128, 1], BF16, tag=f"z{j}")
                nc.vector.tensor_tensor(out=z[:], in0=s_p[:], in1=rec_b[:], op=MULT)
                nc.tensor.matmul(
                    out=x_p[:], lhsT=z[:], rhs=qww[:, 1024 + 256 * j : 1280 + 256 * j],
                    start=False, stop=(j == 1), skip_group_check=True,
                )

            # ---- epilogue: leaky (1 op), norm2 (1 op), sqrt, recip, scale ----
            if has_wb:
                x2 = sb.tile([1, O], F32)
                nc.vector.tensor_tensor(out=x2[:], in0=x_p[:], in1=wb[:], op=ADD)
                xsrc = x2
            else:
                xsrc = x_p
            o2 = sb.tile([1, O], F32)
            nc.scalar.activation(
                out=o2[:], in_=xsrc[:], func=AF.Prelu, alpha=ALPHA
            )
            sq = sb.tile([1, O], F32)
            n2 = sb.tile([1, 1], F32)
            nc.vector.tensor_tensor(out=sq[:], in0=o2[:], in1=o2[:], op=MULT)
            nc.vector.tensor_reduce(out=n2[:], in_=sq[:], axis=AXC, op=ADD)
            nrm = sb.tile([1, 1], F32)
            nc.scalar.activation(out=nrm[:], in_=n2[:], func=AF.Sqrt)
            rc2 = sb.tile([1, 1], F32)
            nc.vector.reciprocal(rc2[:], nrm[:])
            res = sb.tile([1, O], F32)
            nc.vector.tensor_scalar_mul(res[:], o2[:], rc2[:])

            nc.sync.dma_start(out=out_d[:], in_=res[:])

    nc.finalize()
    return nc


@functools.lru_cache(maxsize=4)
def _program(has_qb: bool, has_wb: bool) -> bass.Bass:
    return _build_program(has_qb, has_wb)


def kernel(
    embeddings: np.ndarray,
    weights: np.ndarray,
    Qw: np.ndarray,
    Qb: np.ndarray,
    Ww: np.ndarray,
    Wb: np.ndarray,
    neighbor_set: np.ndarray,
    node_id,
    _trace: bool = False,
):
    import ml_dtypes

    bf16 = ml_dtypes.bfloat16
    node_id = int(np.asarray(node_id))
    nbr = np.asarray(neighbor_set).astype(np.int64).reshape(K)
    emb = np.asarray(embeddings, dtype=np.float32)
    qb_full = np.asarray(Qb, dtype=np.float32).reshape(H)
    wb_full = np.asarray(Wb, dtype=np.float32).reshape(O)
    has_qb = bool(np.any(qb_full))
    has_wb = bool(np.any(wb_full))

    # shared (core-independent) weight tiles
    qw_np = np.asarray(Qw, dtype=np.float32)
    ww_np = np.asarray(Ww, dtype=np.float32)
    qww = np.concatenate(
        [qw_np[0:128, :], qw_np[128:256, :]]
        + [ww_np[128 * j : 128 * (j + 1), :] for j in range(4)],
        axis=1,
    ).astype(bf16)
    wcol = np.asarray(weights[nbr, node_id], dtype=np.float32)  # [K]

    nc = _program(has_qb, has_wb)
    in_maps = []
    for b in range(N_CORES):
        g = emb[b, nbr, :]  # [K, C]
        e_node = emb[b, node_id, :]  # [C]
        gt = np.zeros((128, 256), dtype=np.float32)
        gt[:, 0:64] = g[:, 0:128].T
        gt[:, 64:128] = g[:, 128:256].T
        gt[0:K, 128] = wcol
        gt[:, 132] = e_node[0:128]
        gt[:, 136] = e_node[128:256]
        gt[0, 144:208] = wcol
        m = {"gt": gt.astype(bf16), "qww": qww}
        if has_qb:
            m["qb"] = qb_full.reshape(1, H).astype(bf16)
        if has_wb:
            m["wb"] = np.ascontiguousarray(wb_full.reshape(1, O))
        in_maps.append(m)

    r = run_bass_kernel_spmd(nc, in_maps, list(range(N_CORES)), trace=_trace)
    out = np.stack([r.results[b]["out"][0] for b in range(N_CORES)], axis=0)
    if _trace:
        return out, r
    return out


# revision 13
# speedup vs baseline: 1.3887x; 1.0174x over previous
"""GNN message-passing (Convolve) kernel for Trainium2, 8 NeuronCores.

Reference computation (B=8, N=8192, C=256, H=256, O=256, K=64):
    g   = embeddings[:, neighbor_set, :]                     # [B, K, C]
    h   = leaky_relu(g @ Qw + Qb)                            # [B, K, H]
    w   = weights[neighbor_set, node_id]                     # [K]
    s   = sum_k h * w / (sum_k w + eps)                      # [B, H]
    z   = concat(embeddings[:, node_id, :], s)               # [B, C+H]
    o   = leaky_relu(z @ Ww + Wb)                            # [B, O]
    out = o / (||o||_2 + eps)                                # [B, O]

Sharding: data-parallel over the batch axis — core b handles batch b.
The host performs all *indexing/layout* work (neighbor gather, transpose,
bf16 cast, weight-column extraction); every FLOP of the reference
computation (both matmuls, the weighted mean, the activations, the L2
normalization) runs on device.

Per-core device inputs (all bf16, >=512B per partition line for full DMA
descriptor throughput):
    gt  [128, 256]:  cols 0:64 = g[:, 0:128].T, cols 64:128 = g[:,128:256].T,
                     col 128 = w as a column (K=64 partitions),
                     col 129/130 = node embedding halves (z0 | z1),
                     row 0 cols 136:200 = w as a row (for the DVE den-reduce)
    qwt [128, 512]:  [Qw[0:128, :] | Qw[128:256, :]]
    wwt [128, 1024]: [Ww[0:128,:] | Ww[128:256,:] | Ww[256:384,:] | Ww[384:512,:]]

Device dataflow: the three input DMAs issue in parallel on the sync /
vector / tensor engine queues.  den = sum(w) reduces on DVE, reciprocal,
then a tiny ones-matmul broadcasts 1/den across 128 partitions.  h runs
as 4 bf16 matmuls split by h-column halves so the DVE leaky-relu
(scalar_tensor_tensor: out = max(0.3*x, x), one op) pipelines with the PE.
s = h_l.T @ w_col per 128-chunk; the PSUM->SBUF copy is fused with the
1/den scaling (tensor_tensor mult).  x accumulates in one PSUM group:
z0/z1 (node) and z2/z3 (s) columns against Ww row-blocks.  Epilogue:
leaky on DVE, square+norm2 in one tensor_tensor_reduce, Sqrt on ACT
(table pre-warmed), reciprocal + scale on DVE, contiguous 1KB out DMA.
"""

import functools

import numpy as np

import concourse.bacc as bacc
import concourse.bass as bass
import concourse.mybir as mybir
import concourse.tile as tile
from concourse.bass_utils import run_bass_kernel_spmd

B, N, C, H, O, K = 8, 8192, 256, 256, 256, 64
ALPHA = 0.3
F32 = mybir.dt.float32
BF16 = mybir.dt.bfloat16
N_CORES = 8
MULT = mybir.AluOpType.mult
ADD = mybir.AluOpType.add
MAX = mybir.AluOpType.max
AF = mybir.ActivationFunctionType
AXC = mybir.AxisListType.X


def _build_program(has_qb: bool, has_wb: bool) -> bass.Bass:
    nc = bacc.Bacc(None, target_bir_lowering=False, debug=False)

    gt_d = nc.dram_tensor("gt", [128, 256], BF16, kind="ExternalInput")
    qwt_d = nc.dram_tensor("qwt", [128, 512], BF16, kind="ExternalInput")
    wwt_d = nc.dram_tensor("wwt", [128, 1024], BF16, kind="ExternalInput")
    if has_qb:
        qb_d = nc.dram_tensor("qb", [1, H], BF16, kind="ExternalInput")
    if has_wb:
        wb_d = nc.dram_tensor("wb", [1, O], F32, kind="ExternalInput")
    out_d = nc.dram_tensor("out", [1, O], F32, kind="ExternalOutput")

    with tile.TileContext(nc) as tc:
        with (
            tc.tile_pool(name="sb", bufs=1) as sb,
            tc.tile_pool(name="ps", bufs=1, space="PSUM") as ps,
        ):
            # ---- parallel DMA issue, one input per engine queue ----
            gt = sb.tile([128, 256], BF16)
            nc.sync.dma_start(out=gt[:], in_=gt_d[:])
            qwt = sb.tile([128, 512], BF16)
            nc.scalar.dma_start(out=qwt[:], in_=qwt_d[:])
            wwt = sb.tile([128, 1024], BF16)
            nc.scalar.dma_start(out=wwt[:], in_=wwt_d[:])
            if has_qb:
                qb = sb.tile([1, H], BF16)
                nc.scalar.dma_start(out=qb[:], in_=qb_d[:])
            if has_wb:
                wb = sb.tile([1, O], F32)
                nc.scalar.dma_start(out=wb[:], in_=wb_d[:])

            # ---- constants + ACT table warm (no DMA deps) ----
            ones_m = sb.tile([K, 128], BF16)
            nc.gpsimd.memset(ones_m[:], 1.0)
            if has_qb:
                onesk = sb.tile([1, K], BF16)
                nc.gpsimd.memset(onesk[:], 1.0)
            warm_t = sb.tile([1, 1], F32)
            nc.scalar.activation(out=warm_t[:], in_=ones_m[0:1, 0:1], func=AF.Sqrt)
            warm_p = sb.tile([1, 1], F32)
            nc.scalar.activation(
                out=warm_p[:], in_=ones_m[0:1, 0:1], func=AF.Prelu, alpha=ALPHA
            )

            # ---- den replicated across 128 partitions via ones-matrix
            # matmul (ones[K,128].T @ w = sum(w) per partition), then a
            # per-partition reciprocal straight out of PSUM ----
            den_bp = ps.tile([128, 1], F32, tag="rb")
            nc.tensor.matmul(
                out=den_bp[:], lhsT=ones_m[:], rhs=gt[0:K, 128:129],
                start=True, stop=True, skip_group_check=True,
            )
            rec_b = sb.tile([128, 1], F32)
            nc.vector.reciprocal(rec_b[:], den_bp[:])

            # ---- h = leaky(gT.T @ Qw (+Qb)), split by h-column halves.
            # Separate PSUM tiles per half: each accumulation group gets its
            # own PSUM bank (two open groups in one bank wedge the PE). ----
            h_ps = []
            for j in range(2):
                h_half = ps.tile([K, 128], F32, tag=f"h{j}", name=f"h_half{j}")
                h_ps.append(h_half)
            h_l = sb.tile([K, H], BF16)
            for j in range(2):
                nc.tensor.matmul(
                    out=h_ps[j][:], lhsT=gt[:, 0:64],
                    rhs=qwt[:, 128 * j : 128 * (j + 1)],
                    start=True, stop=False, skip_group_check=True,
                )
                nc.tensor.matmul(
                    out=h_ps[j][:], lhsT=gt[:, 64:128],
                    rhs=qwt[:, 256 + 128 * j : 384 + 128 * j],
                    start=False, stop=not has_qb, skip_group_check=True,
                )
                if has_qb:
                    nc.tensor.matmul(
                        out=h_ps[j][:], lhsT=onesk[:],
                        rhs=qb[:, 128 * j : 128 * (j + 1)],
                        start=False, stop=True, skip_group_check=True,
                    )
                nc.scalar.activation(
                    out=h_l[:, 128 * j : 128 * (j + 1)], in_=h_ps[j][:],
                    func=AF.Prelu, alpha=ALPHA,
                )

            # ---- s chunks on PE back-to-back, scaled copies on DVE ----
            zs = []
            for j in range(2):
                s_p = ps.tile([128, 1], F32, tag=f"s{j}", name=f"s_p{j}")
                nc.tensor.matmul(
                    out=s_p[:], lhsT=h_l[:, 128 * j : 128 * (j + 1)],
                    rhs=gt[0:K, 128:129], start=True, stop=True,
                    skip_group_check=True,
                )
                z = sb.tile([128, 1], BF16, tag=f"z{j}", name=f"z{j}")
                nc.vector.tensor_tensor(out=z[:], in0=s_p[:], in1=rec_b[:], op=MULT)
                zs.append(z)

            # ---- x PSUM group: node part (needs wwt), then s parts ----
            x_p = ps.tile([1, O], F32, tag="x")
            nc.tensor.matmul(
                out=x_p[:], lhsT=gt[:, 132:133], rhs=wwt[:, 0:256],
                start=True, stop=False, skip_group_check=True,
            )
            nc.tensor.matmul(
                out=x_p[:], lhsT=gt[:, 136:137], rhs=wwt[:, 256:512],
                start=False, stop=False, skip_group_check=True,
            )
            for j in range(2):
                nc.tensor.matmul(
                    out=x_p[:], lhsT=zs[j][:],
                    rhs=wwt[:, 512 + 256 * j : 768 + 256 * j],
                    start=False, stop=(j == 1), skip_group_check=True,
                )

            # ---- epilogue: leaky (1 op), norm2 (1 op), sqrt, recip, scale ----
            if has_wb:
                x2 = sb.tile([1, O], F32)
                nc.vector.tensor_tensor(out=x2[:], in0=x_p[:], in1=wb[:], op=ADD)
                xsrc = x2
            else:
                xsrc = x_p
            o2 = sb.tile([1, O], F32)
            nc.scalar.activation(
                out=o2[:], in_=xsrc[:], func=AF.Prelu, alpha=ALPHA
            )
            sq = sb.tile([1, O], F32)
            n2 = sb.tile([1, 1], F32)
            nc.vector.scalar_tensor_tensor(
                out=sq[:], in0=o2[:], scalar=1.0, in1=o2[:],
                op0=MULT, op1=MULT, accum_out=n2[:],
            )
            nrm = sb.tile([1, 1], F32)
            nc.scalar.activation(out=nrm[:], in_=n2[:], func=AF.Sqrt)
            rc2 = sb.tile([1, 1], F32)
            nc.vector.reciprocal(rc2[:], nrm[:])
            res = sb.tile([1, O], F32)
            nc.vector.tensor_scalar_mul(res[:], o2[:], rc2[:])

            nc.sync.dma_start(out=out_d[:], in_=res[:])

    nc.finalize()
    return nc


@functools.lru_cache(maxsize=4)
def _program(has_qb: bool, has_wb: bool) -> bass.Bass:
    return _build_program(has_qb, has_wb)


def kernel(
    embeddings: np.ndarray,
    weights: np.ndarray,
    Qw: np.ndarray,
    Qb: np.ndarray,
    Ww: np.ndarray,
    Wb: np.ndarray,
    neighbor_set: np.ndarray,
    node_id,
    _trace: bool = False,
):
    import ml_dtypes

    bf16 = ml_dtypes.bfloat16
    node_id = int(np.asarray(node_id))
    nbr = np.asarray(neighbor_set).astype(np.int64).reshape(K)
    emb = np.asarray(embeddings, dtype=np.float32)
    qb_full = np.asarray(Qb, dtype=np.float32).reshape(H)
    wb_full = np.asarray(Wb, dtype=np.float32).reshape(O)
    has_qb = bool(np.any(qb_full))
    has_wb = bool(np.any(wb_full))

    # shared (core-independent) weight tiles
    qw_np = np.asarray(Qw, dtype=np.float32)
    ww_np = np.asarray(Ww, dtype=np.float32)
    qwt = np.concatenate([qw_np[0:128, :], qw_np[128:256, :]], axis=1).astype(bf16)
    wwt = np.concatenate(
        [ww_np[128 * j : 128 * (j + 1), :] for j in range(4)], axis=1
    ).astype(bf16)
    wcol = np.asarray(weights[nbr, node_id], dtype=np.float32)  # [K]

    nc = _program(has_qb, has_wb)
    in_maps = []
    for b in range(N_CORES):
        g = emb[b, nbr, :]  # [K, C]
        e_node = emb[b, node_id, :]  # [C]
        gt = np.zeros((128, 256), dtype=np.float32)
        gt[:, 0:64] = g[:, 0:128].T
        gt[:, 64:128] = g[:, 128:256].T
        gt[0:K, 128] = wcol
        gt[:, 132] = e_node[0:128]
        gt[:, 136] = e_node[128:256]
        gt[0, 144:208] = wcol
        m = {"gt": gt.astype(bf16), "qwt": qwt, "wwt": wwt}
        if has_qb:
            m["qb"] = qb_full.reshape(1, H).astype(bf16)
        if has_wb:
            m["wb"] = np.ascontiguousarray(wb_full.reshape(1, O))
        in_maps.append(m)

    r = run_bass_kernel_spmd(nc, in_maps, list(range(N_CORES)), trace=_trace)
    out = np.stack([r.results[b]["out"][0] for b in range(N_CORES)], axis=0)
    if _trace:
        return out, r
    return out
